# revision 10
# baseline (speedup 1.0000x reference)
"""Trainium2 Bass kernel for nn_MAF (masked autoregressive flow forward pass).

Math (per batch row b):
  for layer l in 0..62:
    h1 = tanh(x[:, :l+1] @ W1[l, :l+1] + b1[l])       (masking folded into W1 on host)
    h2 = tanh(h1 @ W2[l] + b2[l])
    (mu_l, alpha_l) = h2 @ W3[l] + b3[l]
  mu = [ip0, mu_0..mu_62]; alpha = [ip1, alpha_0..alpha_62]
  z = (x - mu) * exp(-alpha);  out = (z[:, ::-1], -sum(alpha))

Mapping: pure data-parallel over 8 cores (2048 batch rows each). On-chip
layout is feature-on-partition / batch-on-free. Layers are processed in
pairs: layer 2p on partitions 0:64, layer 2p+1 on partitions 64:128, via a
concatenated W1 stationary [64,128] and block-diagonal W2/W3 [128,128], so
every matmul streams N=512 batch columns at full issue rate and every tanh
runs on all 128 ACT lanes. Matmul operands are float32r (fp32 bytes, PE
1 cycle/row instead of 4; measured output rel-err ~2e-4). Stage 3
accumulates all layers' (mu, alpha) into one PSUM tile per 1024-column
chunk: mu slots on partitions 0:64 (alpha-sum in slot 32, mu of layer l in
slot l<32 ? l : l+1), alpha slots mirrored on partitions 64:128; a
host-permuted copy of x (xs) aligns the epilogue elementwise ops with that
slot order. The epilogue per chunk is one ACT Exp plus four DVE ops and
DMAs. The (chunk, pair) loop is a single continuous software pipeline
(stage2 lags 1 step, stage3 lags 2) so the ACT engine - the bottleneck at
2 x tanh[128,1024] = ~2.3us per step vs ~1.7us of PE work - never stalls.
"""

import numpy as np

B_TOTAL = 16384
DIM = 64
HID = 64
L = 63          # real layers
LP = 64         # padded layer count (layer 63 is all-zero)
NPAIR = 32
NCORES = 8
BC = B_TOTAL // NCORES   # 2048 batch rows per core
FD = 1024                # free-dim (batch) chunk per pipeline step
NCH = BC // FD

_cache: dict = {}


def _qmu(l: int) -> int:
    # partition slot for layer l's mu/alpha (slot 32 is reserved for alpha-sum)
    return l if l < 32 else l + 1


def _build_program(reps: int = 1, npair: int = NPAIR):
    import contextlib
    import concourse.bacc as bacc
    import concourse.mybir as mybir
    import concourse.tile as tile
    from concourse.alu_op_type import AluOpType

    f32 = mybir.dt.float32
    f32r = mybir.dt.float32r
    AF = mybir.ActivationFunctionType

    nc = bacc.Bacc()
    xt = nc.dram_tensor("xt", [DIM, BC], f32r, kind="ExternalInput")
    xs = nc.dram_tensor("xs", [DIM, BC], f32, kind="ExternalInput")
    w1 = nc.dram_tensor("w1", [NPAIR, 64, 128], f32r, kind="ExternalInput")
    w2 = nc.dram_tensor("w2", [NPAIR, 128, 128], f32r, kind="ExternalInput")
    w3 = nc.dram_tensor("w3", [NPAIR, 128, 128], f32r, kind="ExternalInput")
    b1 = nc.dram_tensor("b1", [128, NPAIR], f32, kind="ExternalInput")
    b2 = nc.dram_tensor("b2", [128, NPAIR], f32, kind="ExternalInput")
    bv = nc.dram_tensor("bv", [128, 2], f32, kind="ExternalInput")
    mc = nc.dram_tensor("mc", [64, 4], f32, kind="ExternalInput")
    zt = nc.dram_tensor("zt", [DIM, BC], f32, kind="ExternalOutput")
    ld = nc.dram_tensor("ld", [1, BC], f32, kind="ExternalOutput")

    NV = NCH * npair  # total pipeline steps (chunk-major)

    with tile.TileContext(nc) as tc:
        with (
            tc.tile_pool(name="consts", bufs=1) as cp,
            tc.tile_pool(name="p1", bufs=2, space="PSUM") as pp1,
            tc.tile_pool(name="p2", bufs=1, space="PSUM") as pp2,
            tc.tile_pool(name="p3", bufs=1, space="PSUM") as pp3,
            tc.tile_pool(name="hp", bufs=2) as hp,
            tc.tile_pool(name="zp", bufs=2) as zp,
        ):
            xt_t = cp.tile([DIM, BC], f32r, tag="xt")
            nc.sync.dma_start(xt_t[:], xt[:])
            xs_t = cp.tile([DIM, BC], f32, tag="xs")
            nc.sync.dma_start(xs_t[:], xs[:])
            b1_t = cp.tile([128, NPAIR], f32, tag="b1")
            nc.sync.dma_start(b1_t[:], b1[:])
            b2_t = cp.tile([128, NPAIR], f32, tag="b2")
            nc.sync.dma_start(b2_t[:], b2[:])
            bv_t = cp.tile([128, 2], f32, tag="bv")
            nc.sync.dma_start(bv_t[:], bv[:])
            mc_t = cp.tile([64, 4], f32, tag="mc")
            nc.sync.dma_start(mc_t[:], mc[:])
            w1_t, w2_t, w3_t = [], [], []
            for p in range(npair):
                t1 = cp.tile([64, 128], f32r, tag=f"w1_{p}")
                nc.sync.dma_start(t1[:], w1[p, :, :])
                w1_t.append(t1)
                t2 = cp.tile([128, 128], f32r, tag=f"w2_{p}")
                nc.sync.dma_start(t2[:], w2[p, :, :])
                w2_t.append(t2)
                t3 = cp.tile([128, 128], f32r, tag=f"w3_{p}")
                nc.sync.dma_start(t3[:], w3[p, :, :])
                w3_t.append(t3)

            def epilogue(c, ps3):
                cs = c * FD
                e_t = zp.tile([64, FD], f32, tag="e")
                nc.scalar.activation(e_t[:], ps3[64:128, :], AF.Exp,
                                     bias=bv_t[64:128, 1:2], scale=-1.0)
                nd_t = zp.tile([64, FD], f32, tag="nd")
                nc.vector.scalar_tensor_tensor(
                    nd_t[:], ps3[0:64, :], bv_t[0:64, 0:1], xs_t[:, cs:cs + FD],
                    op0=AluOpType.add, op1=AluOpType.subtract,
                )
                z_t = zp.tile([64, FD], f32, tag="z")
                nc.vector.scalar_tensor_tensor(
                    z_t[:], nd_t[:], -1.0, e_t[:],
                    op0=AluOpType.mult, op1=AluOpType.mult,
                )
                z0_t = zp.tile([1, FD], f32, tag="z0")
                nc.vector.tensor_scalar(
                    z0_t[:], xt_t[0:1, cs:cs + FD].bitcast(f32),
                    mc_t[0:1, 0:1], mc_t[0:1, 1:2],
                    op0=AluOpType.mult, op1=AluOpType.add,
                )
                ld_t = zp.tile([1, FD], f32, tag="ldt")
                nc.vector.tensor_scalar(
                    ld_t[:], ps3[32:33, :], -1.0, mc_t[32:33, 2:3],
                    op0=AluOpType.mult, op1=AluOpType.add,
                )
                nc.sync.dma_start(zt[0:1, cs:cs + FD], z0_t[:])
                nc.sync.dma_start(zt[1:33, cs:cs + FD], z_t[0:32, :])
                nc.sync.dma_start(zt[33:64, cs:cs + FD], z_t[33:64, :])
                nc.sync.dma_start(ld[:, cs:cs + FD], ld_t[:])

            loop_cm = (
                tc.For_i(0, reps, 1, hint_engines=(mybir.EngineType.PE,))
                if reps > 1 else contextlib.nullcontext()
            )
            with loop_cm:
                h1s, h2s, ps3s = {}, {}, {}
                # one continuous software pipeline over all (chunk, pair)
                # steps: stage1(v) | stage2(v-1) | stage3(v-2)
                for vi in range(NV + 2):
                    if vi < NV:
                        c, p = divmod(vi, npair)
                        cs = c * FD
                        ps1 = pp1.tile([128, FD], f32, tag="ps1")
                        for j in range(0, FD, 512):
                            nc.tensor.matmul(
                                ps1[:, j:j + 512], w1_t[p][:],
                                xt_t[0:64, cs + j:cs + j + 512],
                                start=True, stop=True,
                            )
                        h1 = hp.tile([128, FD], f32r, tag="h1")
                        nc.scalar.activation(h1[:], ps1[:], AF.Tanh, bias=b1_t[:, p:p + 1])
                        h1s[vi] = h1
                    w = vi - 1
                    if 0 <= w < NV:
                        c, p = divmod(w, npair)
                        ps2 = pp2.tile([128, FD], f32, tag="ps2")
                        for j in range(0, FD, 512):
                            nc.tensor.matmul(
                                ps2[:, j:j + 512], w2_t[p][:], h1s[w][:, j:j + 512],
                                start=True, stop=True,
                            )
                        h2 = hp.tile([128, FD], f32r, tag="h2")
                        nc.scalar.activation(h2[:], ps2[:], AF.Tanh, bias=b2_t[:, p:p + 1])
                        h2s[w] = h2
                        del h1s[w]
                    w = vi - 2
                    if 0 <= w < NV:
                        c, p = divmod(w, npair)
                        if p == 0:
                            ps3s[c] = pp3.tile([128, FD], f32, tag="ps3", name="ps3")
                        for j in range(0, FD, 512):
                            nc.tensor.matmul(
                                ps3s[c][:, j:j + 512], w3_t[p][:], h2s[w][:, j:j + 512],
                                start=(p == 0), stop=(p == npair - 1),
                                skip_group_check=True,
                            )
                        del h2s[w]
                        if p == npair - 1:
                            epilogue(c, ps3s.pop(c))

    nc.finalize()
    return nc


def _pack(x, initial_param, W1, b1, W2, b2, W3, b3):
    f = np.float32
    x = np.asarray(x, f)
    W1 = np.asarray(W1, f); W2 = np.asarray(W2, f); W3 = np.asarray(W3, f)
    b1 = np.asarray(b1, f); b2 = np.asarray(b2, f); b3 = np.asarray(b3, f)
    ip = np.asarray(initial_param, f)

    mask = (np.arange(L)[None, :] <= np.arange(L)[:, None]).astype(f)
    W1m = W1 * mask[:, :, None]
    W1z = np.zeros((LP, 64, HID), f)
    W1z[:L, :L, :] = W1m
    W1p = np.ascontiguousarray(np.concatenate([W1z[0::2], W1z[1::2]], axis=2))

    W2z = np.zeros((LP, HID, HID), f)
    W2z[:L] = W2
    W2p = np.zeros((NPAIR, 128, 128), f)
    W2p[:, 0:64, 0:64] = W2z[0::2]
    W2p[:, 64:128, 64:128] = W2z[1::2]

    W3z = np.zeros((LP, HID, 2), f)
    W3z[:L] = W3
    W3p = np.zeros((NPAIR, 128, 128), f)
    for l in range(L):
        p, half = divmod(l, 2)
        r = slice(0, 64) if half == 0 else slice(64, 128)
        q = _qmu(l)
        W3p[p, r, q] = W3z[l][:, 0]
        W3p[p, r, 64 + q] = W3z[l][:, 1]
        W3p[p, r, 32] += W3z[l][:, 1]

    b1z = np.zeros((LP, HID), f); b1z[:L] = b1
    b2z = np.zeros((LP, HID), f); b2z[:L] = b2
    b1p = np.ascontiguousarray(np.concatenate([b1z[0::2], b1z[1::2]], axis=1).T)
    b2p = np.ascontiguousarray(np.concatenate([b2z[0::2], b2z[1::2]], axis=1).T)

    bvv = np.zeros((128, 2), f)
    for l in range(L):
        q = _qmu(l)
        bvv[q, 0] = b3[l, 0]
        bvv[64 + q, 1] = -b3[l, 1]

    # z0 = c1*x0 - c2 ; log_det = -alphasum - Cb
    c1 = float(np.exp(-ip[1])); c2 = float(ip[0]) * c1
    Cb = float(b3[:, 1].sum()) + float(ip[1])
    mcv = np.tile(np.array([[c1, -c2, -Cb, 0.0]], f), (64, 1))

    xt_all = np.ascontiguousarray(x.T)
    xs_all = np.zeros((DIM, x.shape[0]), f)
    for l in range(L):
        xs_all[_qmu(l)] = x[:, l + 1]

    return dict(w1=W1p, w2=W2p, w3=W3p, b1=b1p, b2=b2p, bv=bvv, mc=mcv), xt_all, xs_all


def _get_program():
    nc = _cache.get("nc")
    if nc is None:
        nc = _build_program()
        _cache["nc"] = nc
    return nc


def make_in_maps(**inputs):
    shared, xt_all, xs_all = _pack(**inputs)
    in_maps = []
    for c in range(NCORES):
        sl = slice(c * BC, (c + 1) * BC)
        m = dict(shared)
        m["xt"] = np.ascontiguousarray(xt_all[:, sl])
        m["xs"] = np.ascontiguousarray(xs_all[:, sl])
        in_maps.append(m)
    return in_maps


def assemble_output(results):
    zt = np.concatenate([np.asarray(results[c]["zt"]) for c in range(NCORES)], axis=1)
    ldv = np.concatenate([np.asarray(results[c]["ld"]) for c in range(NCORES)], axis=1)[0]
    z = np.ascontiguousarray(zt.T[:, ::-1]).astype(np.float32)
    return z, np.ascontiguousarray(ldv).astype(np.float32)


def kernel(**inputs):
    from concourse import bass_utils
    nc = _get_program()
    in_maps = make_in_maps(**inputs)
    res = bass_utils.run_bass_kernel_spmd(nc, in_maps, core_ids=list(range(NCORES)))
    return assemble_output(res.results)
